# revision 17
# baseline (speedup 1.0000x reference)
"""Distributed causal multi-head attention with cumulative relative-position
bias for Trainium2 (8 NeuronCores).

Problem: x:[2,2048,1024], qkv:[1024,3,16,64], out_w:[16,64,1024],
rpe_bias:[16,2048] -> out:[2,2048,1024]

Sharding: data-parallel over batch (2) x tensor-parallel over head groups
(16 heads -> 4 groups of 4). Core c handles batch c//4, heads [4*(c%4), 4*(c%4)+4).
Each core emits a partial output [2048,1024] (bf16); the host sums the 4
head-group partials per batch (the "all-reduce" of the out projection).

Math tricks:
 - 1/sqrt(D) folded into the Q projection weights on host.
 - bias[i,j] = cumsum(rpe)[i-j] (i>=j, else -inf) is Toeplitz:
   exp(S + bias) = exp(S) * G where G[k_row, q_col] = exp(cum)[q-k] (0 above
   the diagonal). A single [128, 512+2048] bf16 strip per head serves every
   128-row band of the score matrix; the two heads of a pair are packed in
   one [128, 2*(512+2048)] strip so one strided AP covers both.
 - Scores are computed transposed (S^T[k,q] = K_h Q_h^T) so that P^T feeds
   the PV matmul directly (lhsT = V natural layout) with no transposes.
 - A ones-column appended to V yields softmax row-sums for free in the same
   matmul; normalization happens on the SBUF copy of mix^T.

Schedule (v6): attention processes HEAD-PAIRS: per 128-row k-block, the even
head's scores (partitions 0-63 of Q/K) and the odd head's (partitions 64-127)
are computed by two row-tiled 64x128 matmuls (tile rows 0 / 64) that execute
CONCURRENTLY in the PE array, into the two regions of a [128,1024] score
band (region0 = even head, region1 = odd head; different PSUM banks as
row-tiling requires).  One exp per band (stale/hole columns are finite --
the band PSUM pool is zeroed once -- and get G=0), one G-multiply via the
packed dual-head strided AP, then two serial PV matmuls (M=65, ones-column
rowsum).  mix PSUM is evicted to SBUF with a single [65,512] copy (same DVE
cost as the old [1,512] rowsum copy) so each mix bank recycles immediately
instead of living through the reciprocal chain.  Projection / out-projection
matmuls are interleaved as FILLER STEPS inside the attention loop.
Input DMAs are spread across the GpSimd/Tensor/Sync/Vector/Scalar queues in
consumption order so the first projection starts ~5us in; output DMAs are
split per 512-column half and alternate the Sync/GpSimd queues.
"""

import sys

if "/opt/trn_rl_repo" not in sys.path:
    sys.path.insert(0, "/opt/trn_rl_repo")

import numpy as np
import ml_dtypes

B, S, HID, NH, D = 2, 2048, 1024, 16, 64
NCORES = 8
HPC = 4  # heads per core
KB = 16  # 128-row k blocks
GW = 512 + S  # per-head G strip width
BF16 = ml_dtypes.bfloat16

_CACHE = {}


def build_nc():
    import concourse.mybir as mybir
    from concourse import bacc
    from concourse.tile import TileContext
    from concourse.ap import AP

    f32 = mybir.dt.float32
    bf16 = mybir.dt.bfloat16
    Exp = mybir.ActivationFunctionType.Exp
    Copy = mybir.ActivationFunctionType.Copy

    nc = bacc.Bacc()

    x_t = nc.declare_dram_parameter("x_t", [HID, S], bf16, isOutput=False)
    w_qk = nc.declare_dram_parameter("w_qk", [HID, 512], bf16, isOutput=False)
    w_v = nc.declare_dram_parameter("w_v", [HID, 260], bf16, isOutput=False)
    g_ext = nc.declare_dram_parameter("g_ext", [2, 128, 2 * GW], bf16, isOutput=False)
    w_out = nc.declare_dram_parameter("w_out", [256, HID], bf16, isOutput=False)
    out = nc.declare_dram_parameter("out", [S, HID], bf16, isOutput=True)

    with TileContext(nc) as tc:
        with (
            tc.tile_pool(name="persist", bufs=1) as persist,
            tc.tile_pool(name="work", bufs=3) as work,
            tc.tile_pool(name="work2", bufs=2) as work2,
            tc.tile_pool(name="msbp", bufs=3) as msbp,
            tc.tile_pool(name="dram", bufs=2, space="DRAM") as dpool,
            tc.tile_pool(name="psum", bufs=2, space="PSUM") as psum,
        ):
            # warm the exp activation-table during the input DMA wait
            warm = persist.tile([1, 8], f32, tag="warm", name="warm")
            nc.vector.memset(warm, 0.0)
            nc.scalar.activation(out=warm, in_=warm, func=Exp)

            # ---- input DMAs: 3 queues (GpSimd/Sync/Scalar), consumption order
            # First projection pass (qk_pass mt=0,2) needs all wqk tiles +
            # x cols [0,1024); V st0-7 needs x cols [0,1024).  Queues:
            #   GpSimd: wqk 0-3, x0-3 column blocks in order, wout
            #   Sync:   wqk 4-7, x4-7 column blocks in order, g2[1] 2nd half
            #   Scalar: wv 0-7, g2[0], g2[1] first half
            wqk_sb, xt_sb = [], []
            for i in range(8):
                tw = persist.tile([128, 512], bf16, tag=f"wqk{i}", name=f"wqk{i}")
                wqk_sb.append(tw)
                tx = persist.tile([128, S], bf16, tag=f"xt{i}", name=f"xt{i}")
                xt_sb.append(tx)
            g_sb = [
                persist.tile([128, 2 * GW], bf16, tag=f"g{hp}", name=f"g{hp}")
                for hp in range(2)
            ]
            wv_sb = [
                persist.tile([128, 260], bf16, tag=f"wv{i}", name=f"wv{i}")
                for i in range(8)
            ]
            wout_sb = [
                persist.tile([128, HID], bf16, tag=f"wout{i}", name=f"wout{i}")
                for i in range(2)
            ]
            for i in range(4):
                nc.gpsimd.dma_start(out=wqk_sb[i], in_=w_qk[i * 128 : (i + 1) * 128, :])
                nc.sync.dma_start(
                    out=wqk_sb[4 + i], in_=w_qk[(4 + i) * 128 : (5 + i) * 128, :]
                )
            for c in range(2):
                for i in range(4):
                    nc.gpsimd.dma_start(
                        out=xt_sb[i][:, 512 * c : 512 * (c + 1)],
                        in_=x_t[i * 128 : (i + 1) * 128, 512 * c : 512 * (c + 1)],
                    )
                    nc.sync.dma_start(
                        out=xt_sb[4 + i][:, 512 * c : 512 * (c + 1)],
                        in_=x_t[(4 + i) * 128 : (5 + i) * 128, 512 * c : 512 * (c + 1)],
                    )
            for i in range(8):
                nc.scalar.dma_start(out=wv_sb[i], in_=w_v[i * 128 : (i + 1) * 128, :])
            nc.scalar.dma_start(out=g_sb[0], in_=g_ext[0])

            def x_second_wave():
                # issued after the first projection passes are emitted so the
                # transfers queue behind the startup-critical ones.
                for c in range(2, 4):
                    for i in range(4):
                        nc.gpsimd.dma_start(
                            out=xt_sb[i][:, 512 * c : 512 * (c + 1)],
                            in_=x_t[i * 128 : (i + 1) * 128, 512 * c : 512 * (c + 1)],
                        )
                        nc.sync.dma_start(
                            out=xt_sb[4 + i][:, 512 * c : 512 * (c + 1)],
                            in_=x_t[(4 + i) * 128 : (5 + i) * 128, 512 * c : 512 * (c + 1)],
                        )
                nc.sync.dma_start(out=g_sb[1][:, 0:GW], in_=g_ext[1][:, 0:GW])
                nc.gpsimd.dma_start(out=g_sb[1][:, GW : 2 * GW], in_=g_ext[1][:, GW : 2 * GW])
                for i in range(2):
                    nc.gpsimd.dma_start(
                        out=wout_sb[i], in_=w_out[i * 128 : (i + 1) * 128, :]
                    )

            qk_sb = [persist.tile([128, S], bf16, tag=f"qk{mt}", name=f"qk{mt}") for mt in range(4)]
            v_sb = [persist.tile([128, 260], bf16, tag=f"v{st}", name=f"v{st}") for st in range(KB)]
            for st in range(KB):
                ones_cols = v_sb[st].rearrange("p (h c) -> p h c", c=65)
                nc.vector.memset(ones_cols[:, :, 64:65], 1.0)
            mixT_sb = [persist.tile([128, S], bf16, tag=f"mixT{i}", name=f"mixT{i}") for i in range(2)]

            # pexp buffers rotate; stale prefix columns are multiplied by G=0,
            # so they only need to hold FINITE values -> zero them once.
            for i in range(3):
                t = work.tile([128, 1024], bf16, tag="pexp", name="pexpinit")
                nc.vector.memset(t, 0.0)
            # the score-band PSUM pool is exp'ed across hole columns the QK
            # matmuls did not write this rotation; zero it once so stale bits
            # are never NaN/Inf patterns.
            for i in range(2):
                t = psum.tile([128, 1024], f32, tag="sband", name="sbinit", bufs=2)
                nc.vector.memset(t, 0.0)

            # identity + ones for the PE-based final normalization chain
            from concourse.masks import make_identity

            ident128 = persist.tile([128, 128], f32, tag="ident", name="ident")
            make_identity(nc, ident128)
            ones64 = persist.tile([1, 64], f32, tag="ones64", name="ones64")
            nc.vector.memset(ones64, 1.0)

            state = {"chain": 0, "odma": 0}

            def qk_pass(mt, half, evict_act=False):
                """Generator: one yield per matmul; eviction per 512-half."""
                for c in range(2):
                    ps = psum.tile([128, 512], f32, tag="fill", name="qkps", bufs=2)
                    for xc in range(8):
                        nc.tensor.matmul(
                            ps,
                            lhsT=wqk_sb[xc][:, mt * 128 : (mt + 1) * 128],
                            rhs=xt_sb[xc][:, half * 1024 + c * 512 : half * 1024 + (c + 1) * 512],
                            start=(xc == 0),
                            stop=(xc == 7),
                        )
                        yield
                    dst = qk_sb[mt][:, half * 1024 + c * 512 : half * 1024 + (c + 1) * 512]
                    if evict_act:
                        nc.scalar.activation(out=dst, in_=ps, func=Copy)
                    else:
                        nc.vector.tensor_copy(out=dst, in_=ps)

            def v_pass(st):
                ps = psum.tile([128, 512], f32, tag="fill", name="vps", bufs=2)[:, :260]
                for xc in range(8):
                    nc.tensor.matmul(
                        ps,
                        lhsT=xt_sb[xc][:, st * 128 : (st + 1) * 128],
                        rhs=wv_sb[xc],
                        start=(xc == 0),
                        stop=(xc == 7),
                    )
                    yield
                # strided eviction skips the ones columns (memset once at
                # startup; the projection would write zeros over them)
                v3 = v_sb[st].rearrange("p (h c) -> p h c", c=65)
                ps3 = ps.rearrange("p (h c) -> p h c", c=65)
                nc.vector.tensor_copy(out=v3[:, :, 0:64], in_=ps3[:, :, 0:64])

            def out_proj(qb, evict_act=False):
                o_sb = work2.tile([128, HID], bf16, tag="osb", name="osb", bufs=3)
                for nn in range(2):
                    ps = psum.tile([128, 512], f32, tag="fill", name="outps", bufs=2)
                    for hd in range(2):
                        nc.tensor.matmul(
                            ps,
                            lhsT=mixT_sb[hd][:, qb * 128 : (qb + 1) * 128],
                            rhs=wout_sb[hd][:, nn * 512 : (nn + 1) * 512],
                            start=(hd == 0),
                            stop=(hd == 1),
                        )
                        yield
                    dst = o_sb[:, nn * 512 : (nn + 1) * 512]
                    if evict_act:
                        nc.scalar.activation(out=dst, in_=ps, func=Copy)
                    else:
                        nc.vector.tensor_copy(out=dst, in_=ps)
                    qs_rot = (nc.sync, nc.gpsimd, nc.scalar) if evict_act else (nc.sync, nc.gpsimd)
                    odma = qs_rot[state["odma"] % len(qs_rot)]
                    state["odma"] += 1
                    odma.dma_start(
                        out=out[qb * 128 : (qb + 1) * 128, nn * 512 : (nn + 1) * 512],
                        in_=o_sb[:, nn * 512 : (nn + 1) * 512],
                    )

            fillers = []

            def enqueue(gen):
                fillers.append(gen)

            def fill_step(n):
                for _ in range(n):
                    while fillers:
                        try:
                            next(fillers[0])
                            break
                        except StopIteration:
                            fillers.pop(0)
                    else:
                        return

            def run_block(gen):
                for _ in gen:
                    pass

            def flush():
                while fillers:
                    run_block(fillers.pop(0))

            def attention(hp, qs, pops=1):
                """One head-pair (heads 2hp, 2hp+1) x one 512-col q group."""
                he, ho = 2 * hp, 2 * hp + 1
                qt, kt = qk_sb[hp], qk_sb[2 + hp]
                gt = g_sb[hp]
                kbmax = qs // 128 + 3
                mix_o = psum.tile([65, 512], f32, tag="mix", name="mixo", bufs=2)
                mix_e = psum.tile([65, 512], f32, tag="mix", name="mixe", bufs=2)

                def pv(kb, pband):
                    # PV: interior k-blocks skip their zero q-prefix; first and
                    # last stay full-width so PSUM start/stop cover everything.
                    p_ = max(0, 128 * kb - qs)
                    pvs = 0 if kb in (0, kbmax) else p_
                    nc.tensor.matmul(
                        mix_e[:, pvs:],
                        lhsT=v_sb[kb][:, 65 * he : 65 * he + 65],
                        rhs=pband[:, pvs:512],
                        start=(kb == 0),
                        stop=(kb == kbmax),
                    )
                    nc.tensor.matmul(
                        mix_o[:, pvs:],
                        lhsT=v_sb[kb][:, 65 * ho : 65 * ho + 65],
                        rhs=pband[:, 512 + pvs : 1024],
                        start=(kb == 0),
                        stop=(kb == kbmax),
                    )

                pend = None  # (kb, pband) whose PV is deferred one step
                for kb in range(kbmax + 1):
                    k0 = 128 * kb
                    p_ = max(0, k0 - qs)
                    # two-region score band: region0 (cols 0-511) = even head,
                    # region1 (cols 512-1023) = odd head; the 64x128 row-tiled
                    # matmuls (tile rows 0 / 64) execute concurrently.
                    sb = psum.tile([128, 1024], f32, tag="sband", name="sband", bufs=2)
                    nc.tensor.matmul(
                        sb[:, p_:512],
                        lhsT=kt[0:64, k0 : k0 + 128],
                        rhs=qt[0:64, qs + p_ : qs + 512],
                        start=True,
                        stop=True,
                    )
                    nc.tensor.matmul(
                        sb[:, 512 + p_ : 1024],
                        lhsT=kt[64:128, k0 : k0 + 128],
                        rhs=qt[64:128, qs + p_ : qs + 512],
                        start=True,
                        stop=True,
                    )
                    pexp = work.tile([128, 1024], bf16, tag="pexp", name="pexp")
                    nc.scalar.activation(out=pexp[:, p_:1024], in_=sb[:, p_:1024], func=Exp)
                    # previous step's PV goes into the PE FIFO *here* -- its
                    # pband is already ready, so the PE never stalls on it and
                    # the next QK (and therefore the Act exp stream) is never
                    # blocked behind this step's exp->Gmul round trip.
                    if pend is not None:
                        pv(*pend)
                    fill_step(pops)
                    # one G-multiply for both heads: slot a=0 -> even head's
                    # strip, a=1 -> odd head's strip (g_sb packs [even|odd]).
                    o0 = 512 + qs - k0
                    gv = AP(
                        tensor=gt.tensor,
                        offset=gt.offset + o0,
                        ap=[list(gt.ap[0]), [GW, 2], [1, 512]],
                    )
                    pband = work.tile([128, 1024], bf16, tag="pband", name="pband")
                    pb3 = pband.rearrange("p (a b) -> p a b", b=512)
                    px3 = pexp.rearrange("p (a b) -> p a b", b=512)
                    nc.vector.tensor_mul(pb3, px3, gv)
                    fill_step(1)
                    pend = (kb, pband)
                pv(*pend)
                # close both heads: evict mix PSUM to SBUF (frees the bank),
                # then reciprocal-normalize rowsums into mixT.
                for h, mixp in ((he, mix_e), (ho, mix_o)):
                    ms = msbp.tile([65, 512], f32, tag="msb", name="msb")
                    nc.vector.tensor_copy(out=ms, in_=mixp)
                    if h == 3 and qs == 1536:
                        # Final chain gates the kernel tail: route it through
                        # the (idle by now) PE instead of DRAM DMA latency.
                        row_sb = work2.tile([1, 512], f32, tag="row", name="row")
                        nc.vector.tensor_copy(out=row_sb, in_=ms[64:65, :])
                        tp_ps = psum.tile([128, 1024], f32, tag="sband", name="tp", bufs=2)
                        for j in range(4):
                            nc.tensor.transpose(
                                tp_ps[:, j : j + 1],
                                row_sb[:, 128 * j : 128 * (j + 1)],
                                ident128[0:1, 0:1],
                            )
                        rc_sb = work2.tile([128, 4], f32, tag="rc", name="rc")
                        nc.vector.reciprocal(out=rc_sb, in_=tp_ps[:, 0:4])
                        ut_ps = psum.tile([128, 1024], f32, tag="sband", name="ut", bufs=2)
                        for j in range(4):
                            nc.tensor.transpose(
                                ut_ps[0:1, 128 * j : 128 * (j + 1)],
                                rc_sb[:, j : j + 1],
                                ident128,
                            )
                        rrow_sb = work2.tile([1, 512], f32, tag="rrow", name="rrow")
                        nc.vector.tensor_copy(out=rrow_sb, in_=ut_ps[0:1, 0:512])
                        r_ps = psum.tile([128, 1024], f32, tag="sband", name="rps", bufs=2)
                        nc.tensor.matmul(
                            r_ps[0:64, 0:512], lhsT=ones64, rhs=rrow_sb, start=True, stop=True
                        )
                        r_sb = work2.tile([64, 512], f32, tag="rbc", name="rbc")
                        nc.vector.tensor_copy(out=r_sb, in_=r_ps[0:64, 0:512])
                    else:
                        # DRAM bounce wraps [1,512] to [128,4] so the
                        # reciprocal runs on 128 lanes. Chains alternate
                        # SP/GpSimd DMA queues so consecutive heads overlap.
                        dma_eng = nc.sync if state["chain"] % 2 == 0 else nc.gpsimd
                        state["chain"] += 1
                        d_s = dpool.tile([1, 512], f32, tag="ds", name="ds")
                        dma_eng.dma_start(out=d_s, in_=ms[64:65, :])
                        rs_sb = work2.tile([128, 4], f32, tag="rs", name="rs")
                        dma_eng.dma_start(out=rs_sb, in_=d_s.rearrange("o (a b) -> (o a) b", a=128))
                        rc_sb = work2.tile([128, 4], f32, tag="rc", name="rc")
                        nc.vector.reciprocal(out=rc_sb, in_=rs_sb)
                        d_r = dpool.tile([1, 512], f32, tag="dr", name="dr")
                        dma_eng.dma_start(out=d_r.rearrange("o (a b) -> (o a) b", a=128), in_=rc_sb)
                        r_sb = work2.tile([64, 512], f32, tag="rbc", name="rbc")
                        dma_eng.dma_start(out=r_sb, in_=d_r.to_broadcast([64, 512]))
                    # 64-partition DVE ops may write either partition half
                    # (bank->quadrant routing), so odd heads write rows 64-127
                    pb = 64 * (h % 2)
                    nc.vector.tensor_mul(
                        mixT_sb[hp][pb : pb + 64, qs : qs + 512], ms[0:64, :], r_sb
                    )

            # ---------------- schedule (qs-major, per-step fillers) ----------
            # flush() before every segment that consumes queued filler output
            run_block(qk_pass(0, 0, evict_act=True))
            run_block(qk_pass(2, 0, evict_act=True))
            x_second_wave()
            for st in range(4):
                run_block(v_pass(st))
            enqueue(v_pass(4))
            enqueue(v_pass(5))
            attention(0, 0, pops=2)
            flush()
            run_block(qk_pass(1, 0, evict_act=True))
            run_block(qk_pass(3, 0, evict_act=True))
            enqueue(v_pass(6))
            enqueue(v_pass(7))
            attention(1, 0, pops=2)
            flush()  # V0-7 complete for qs=512
            for st in range(8, 12):
                enqueue(v_pass(st))
            enqueue(qk_pass(0, 1))
            enqueue(qk_pass(2, 1))
            attention(0, 512, pops=2)
            attention(1, 512, pops=2)
            flush()  # V8-11 + qk halves complete for qs=1024 (hp=0)
            enqueue(qk_pass(1, 1))
            enqueue(qk_pass(3, 1))
            attention(0, 1024, pops=2)
            flush()  # hp=1 qk halves complete
            for st in range(12, 16):
                enqueue(v_pass(st))
            attention(1, 1024, pops=2)
            flush()  # V12-15 complete for qs=1536
            for qb in range(0, 8):
                enqueue(out_proj(qb))
            attention(0, 1536, pops=2)
            attention(1, 1536, pops=2)
            # out_proj 8-11 held back: they keep the PE warm through the
            # final normalization chains (and their DMAs drain early).
            for qb in range(8, 12):
                enqueue(out_proj(qb))
            flush()
            for qb in range(12, 16):
                run_block(out_proj(qb, evict_act=True))
    nc.finalize()
    return nc


def host_prep(x, qkv, out_w, rpe_bias):
    """Build per-core input shards (all host work is O(N*S) or a transpose)."""
    x = np.asarray(x, np.float32)
    qkv = np.asarray(qkv, np.float32)
    out_w = np.asarray(out_w, np.float32)
    rpe_bias = np.asarray(rpe_bias, np.float32)

    xT = [np.ascontiguousarray(x[b].T).astype(BF16) for b in range(B)]  # [HID,S]

    shards = []
    for hg in range(4):
        hs = slice(hg * 4, hg * 4 + 4)
        wq = qkv[:, 0, hs, :].reshape(HID, 256) * (D ** -0.5)
        wk = qkv[:, 1, hs, :].reshape(HID, 256)
        w_qk = np.concatenate([wq, wk], axis=1).astype(BF16)

        w_v = np.zeros((HID, 260), np.float32)
        for i in range(4):
            w_v[:, i * 65 : i * 65 + 64] = qkv[:, 2, hg * 4 + i, :]
        w_v = w_v.astype(BF16)

        # per-head G strips packed per head-PAIR: [even | odd]
        g = np.zeros((HPC, 128, GW), np.float32)
        idx = np.arange(GW)[None, :] - 512 - np.arange(128)[:, None]
        valid = (idx >= 0) & (idx < S)
        for i in range(4):
            cum = np.cumsum(rpe_bias[hg * 4 + i])
            gh = np.exp(cum)
            g[i] = np.where(valid, gh[np.clip(idx, 0, S - 1)], 0.0)
        g2 = np.concatenate(
            [
                np.concatenate([g[0], g[1]], axis=1)[None],
                np.concatenate([g[2], g[3]], axis=1)[None],
            ]
        ).astype(BF16)

        w_o = out_w[hs].reshape(256, HID).astype(BF16)
        shards.append((w_qk, w_v, g2, w_o))

    in_maps = []
    for c in range(NCORES):
        b, hg = c // 4, c % 4
        w_qk, w_v, g2, w_o = shards[hg]
        in_maps.append(
            {"x_t": xT[b], "w_qk": w_qk, "w_v": w_v, "g_ext": g2, "w_out": w_o}
        )
    return in_maps


def run(in_maps, trace=False):
    from concourse.bass_utils import run_bass_kernel_spmd

    if "nc" not in _CACHE:
        _CACHE["nc"] = build_nc()
    nc = _CACHE["nc"]
    res = run_bass_kernel_spmd(nc, in_maps, core_ids=list(range(NCORES)), trace=trace)
    return res


def kernel(x, qkv, out_w, rpe_bias):
    in_maps = host_prep(x, qkv, out_w, rpe_bias)
    res = run(in_maps)
    parts = [np.asarray(res.results[c]["out"], np.float32) for c in range(NCORES)]
    out = np.stack(
        [
            parts[0] + parts[1] + parts[2] + parts[3],
            parts[4] + parts[5] + parts[6] + parts[7],
        ]
    ).astype(np.float32)
    return out


if __name__ == "__main__":
    nc = build_nc()
    print("built ok")


# revision 19
# speedup vs baseline: 1.0751x; 1.0751x over previous
"""Distributed causal multi-head attention with cumulative relative-position
bias for Trainium2 (8 NeuronCores).

Problem: x:[2,2048,1024], qkv:[1024,3,16,64], out_w:[16,64,1024],
rpe_bias:[16,2048] -> out:[2,2048,1024]

Sharding: data-parallel over batch (2) x tensor-parallel over head groups
(16 heads -> 4 groups of 4). Core c handles batch c//4, heads [4*(c%4), 4*(c%4)+4).
Each core emits a partial output [2048,1024] (bf16); the host sums the 4
head-group partials per batch (the "all-reduce" of the out projection).

Math tricks:
 - 1/sqrt(D) folded into the Q projection weights on host.
 - bias[i,j] = cumsum(rpe)[i-j] (i>=j, else -inf) is Toeplitz:
   exp(S + bias) = exp(S) * G where G[k_row, q_col] = exp(cum)[q-k] (0 above
   the diagonal). A single [128, 512+2048] bf16 strip per head serves every
   128-row band of the score matrix; the two heads of a pair are packed in
   one [128, 2*(512+2048)] strip so one strided AP covers both.
 - Scores are computed transposed (S^T[k,q] = K_h Q_h^T) so that P^T feeds
   the PV matmul directly (lhsT = V natural layout) with no transposes.
 - A ones-column appended to V yields softmax row-sums for free in the same
   matmul; normalization happens on the SBUF copy of mix^T.

Schedule (v6): attention processes HEAD-PAIRS: per 128-row k-block, the even
head's scores (partitions 0-63 of Q/K) and the odd head's (partitions 64-127)
are computed by two row-tiled 64x128 matmuls (tile rows 0 / 64) that execute
CONCURRENTLY in the PE array, into the two regions of a [128,1024] score
band (region0 = even head, region1 = odd head; different PSUM banks as
row-tiling requires).  One exp per band (stale/hole columns are finite --
the band PSUM pool is zeroed once -- and get G=0), one G-multiply via the
packed dual-head strided AP, then two serial PV matmuls (M=65, ones-column
rowsum).  mix PSUM is evicted to SBUF with a single [65,512] copy (same DVE
cost as the old [1,512] rowsum copy) so each mix bank recycles immediately
instead of living through the reciprocal chain.  Projection / out-projection
matmuls are interleaved as FILLER STEPS inside the attention loop.
Input DMAs are spread across the GpSimd/Tensor/Sync/Vector/Scalar queues in
consumption order so the first projection starts ~5us in; output DMAs are
split per 512-column half and alternate the Sync/GpSimd queues.
"""

import sys

if "/opt/trn_rl_repo" not in sys.path:
    sys.path.insert(0, "/opt/trn_rl_repo")

import numpy as np
import ml_dtypes

B, S, HID, NH, D = 2, 2048, 1024, 16, 64
NCORES = 8
HPC = 4  # heads per core
KB = 16  # 128-row k blocks
GW = 512 + S  # per-head G strip width
BF16 = ml_dtypes.bfloat16

_CACHE = {}


def build_nc():
    import concourse.mybir as mybir
    from concourse import bacc
    from concourse.tile import TileContext
    from concourse.ap import AP

    f32 = mybir.dt.float32
    bf16 = mybir.dt.bfloat16
    Exp = mybir.ActivationFunctionType.Exp
    Copy = mybir.ActivationFunctionType.Copy

    nc = bacc.Bacc()

    x_t = nc.declare_dram_parameter("x_t", [HID, S], bf16, isOutput=False)
    w_qk = nc.declare_dram_parameter("w_qk", [HID, 512], bf16, isOutput=False)
    w_v = nc.declare_dram_parameter("w_v", [HID, 260], bf16, isOutput=False)
    g_ext = nc.declare_dram_parameter("g_ext", [2, 128, 2 * GW], bf16, isOutput=False)
    w_out = nc.declare_dram_parameter("w_out", [256, HID], bf16, isOutput=False)
    out = nc.declare_dram_parameter("out", [S, HID], bf16, isOutput=True)

    with TileContext(nc) as tc:
        with (
            tc.tile_pool(name="persist", bufs=1) as persist,
            tc.tile_pool(name="work", bufs=3) as work,
            tc.tile_pool(name="work2", bufs=2) as work2,
            tc.tile_pool(name="msbp", bufs=3) as msbp,
            tc.tile_pool(name="dram", bufs=2, space="DRAM") as dpool,
            tc.tile_pool(name="psum", bufs=2, space="PSUM") as psum,
        ):
            # warm the exp activation-table during the input DMA wait
            warm = persist.tile([1, 8], f32, tag="warm", name="warm")
            nc.vector.memset(warm, 0.0)
            nc.scalar.activation(out=warm, in_=warm, func=Exp)

            # ---- input DMAs: 3 queues (GpSimd/Sync/Scalar), consumption order
            # First projection pass (qk_pass mt=0,2) needs all wqk tiles +
            # x cols [0,1024); V st0-7 needs x cols [0,1024).  Queues:
            #   GpSimd: wqk 0-3, x0-3 column blocks in order, wout
            #   Sync:   wqk 4-7, x4-7 column blocks in order, g2[1] 2nd half
            #   Scalar: wv 0-7, g2[0], g2[1] first half
            wqk_sb, xt_sb = [], []
            for i in range(8):
                tw = persist.tile([128, 512], bf16, tag=f"wqk{i}", name=f"wqk{i}")
                wqk_sb.append(tw)
                tx = persist.tile([128, S], bf16, tag=f"xt{i}", name=f"xt{i}")
                xt_sb.append(tx)
            g_sb = [
                persist.tile([128, 2 * GW], bf16, tag=f"g{hp}", name=f"g{hp}")
                for hp in range(2)
            ]
            wv_sb = [
                persist.tile([128, 260], bf16, tag=f"wv{i}", name=f"wv{i}")
                for i in range(8)
            ]
            wout_sb = [
                persist.tile([128, HID], bf16, tag=f"wout{i}", name=f"wout{i}")
                for i in range(2)
            ]
            for i in range(4):
                nc.gpsimd.dma_start(out=wqk_sb[i], in_=w_qk[i * 128 : (i + 1) * 128, :])
                nc.sync.dma_start(
                    out=wqk_sb[4 + i], in_=w_qk[(4 + i) * 128 : (5 + i) * 128, :]
                )
            for c in range(2):
                for i in range(4):
                    nc.gpsimd.dma_start(
                        out=xt_sb[i][:, 512 * c : 512 * (c + 1)],
                        in_=x_t[i * 128 : (i + 1) * 128, 512 * c : 512 * (c + 1)],
                    )
                    nc.sync.dma_start(
                        out=xt_sb[4 + i][:, 512 * c : 512 * (c + 1)],
                        in_=x_t[(4 + i) * 128 : (5 + i) * 128, 512 * c : 512 * (c + 1)],
                    )
            for i in range(8):
                nc.scalar.dma_start(out=wv_sb[i], in_=w_v[i * 128 : (i + 1) * 128, :])
            nc.scalar.dma_start(out=g_sb[0], in_=g_ext[0])

            def x_second_wave():
                # issued after the first projection passes are emitted so the
                # transfers queue behind the startup-critical ones.
                for c in range(2, 4):
                    for i in range(4):
                        nc.gpsimd.dma_start(
                            out=xt_sb[i][:, 512 * c : 512 * (c + 1)],
                            in_=x_t[i * 128 : (i + 1) * 128, 512 * c : 512 * (c + 1)],
                        )
                        nc.sync.dma_start(
                            out=xt_sb[4 + i][:, 512 * c : 512 * (c + 1)],
                            in_=x_t[(4 + i) * 128 : (5 + i) * 128, 512 * c : 512 * (c + 1)],
                        )
                nc.sync.dma_start(out=g_sb[1][:, 0:GW], in_=g_ext[1][:, 0:GW])
                nc.gpsimd.dma_start(out=g_sb[1][:, GW : 2 * GW], in_=g_ext[1][:, GW : 2 * GW])
                for i in range(2):
                    nc.gpsimd.dma_start(
                        out=wout_sb[i], in_=w_out[i * 128 : (i + 1) * 128, :]
                    )

            qk_sb = [persist.tile([128, S], bf16, tag=f"qk{mt}", name=f"qk{mt}") for mt in range(4)]
            v_sb = [persist.tile([128, 260], bf16, tag=f"v{st}", name=f"v{st}") for st in range(KB)]
            for st in range(KB):
                ones_cols = v_sb[st].rearrange("p (h c) -> p h c", c=65)
                nc.vector.memset(ones_cols[:, :, 64:65], 1.0)
            mixT_sb = [persist.tile([128, S], bf16, tag=f"mixT{i}", name=f"mixT{i}") for i in range(2)]

            # pexp buffers rotate; stale prefix columns are multiplied by G=0,
            # so they only need to hold FINITE values -> zero them once.
            for i in range(3):
                t = work.tile([128, 1024], bf16, tag="pexp", name="pexpinit")
                nc.vector.memset(t, 0.0)
            # the score-band PSUM pool is exp'ed across hole columns the QK
            # matmuls did not write this rotation; zero it once so stale bits
            # are never NaN/Inf patterns.
            for i in range(2):
                t = psum.tile([128, 1024], f32, tag="sband", name="sbinit", bufs=2)
                nc.vector.memset(t, 0.0)

            # identity + ones for the PE-based final normalization chain
            from concourse.masks import make_identity

            ident128 = persist.tile([128, 128], f32, tag="ident", name="ident")
            make_identity(nc, ident128)
            ones64 = persist.tile([1, 64], f32, tag="ones64", name="ones64")
            nc.vector.memset(ones64, 1.0)

            state = {"chain": 0, "odma": 0}

            def qk_pass(mt, half, evict_act=False):
                """Generator: one yield per matmul; eviction per 512-half."""
                for c in range(2):
                    ps = psum.tile([128, 512], f32, tag="fill", name="qkps", bufs=2)
                    for xc in range(8):
                        nc.tensor.matmul(
                            ps,
                            lhsT=wqk_sb[xc][:, mt * 128 : (mt + 1) * 128],
                            rhs=xt_sb[xc][:, half * 1024 + c * 512 : half * 1024 + (c + 1) * 512],
                            start=(xc == 0),
                            stop=(xc == 7),
                        )
                        yield
                    dst = qk_sb[mt][:, half * 1024 + c * 512 : half * 1024 + (c + 1) * 512]
                    if evict_act:
                        nc.scalar.activation(out=dst, in_=ps, func=Copy)
                    else:
                        nc.vector.tensor_copy(out=dst, in_=ps)

            def v_pass(st):
                ps = psum.tile([128, 512], f32, tag="fill", name="vps", bufs=2)[:, :260]
                for xc in range(8):
                    nc.tensor.matmul(
                        ps,
                        lhsT=xt_sb[xc][:, st * 128 : (st + 1) * 128],
                        rhs=wv_sb[xc],
                        start=(xc == 0),
                        stop=(xc == 7),
                    )
                    yield
                # strided eviction skips the ones columns (memset once at
                # startup; the projection would write zeros over them)
                v3 = v_sb[st].rearrange("p (h c) -> p h c", c=65)
                ps3 = ps.rearrange("p (h c) -> p h c", c=65)
                nc.vector.tensor_copy(out=v3[:, :, 0:64], in_=ps3[:, :, 0:64])

            def out_proj(qb, evict_act=False):
                o_sb = work2.tile([128, HID], bf16, tag="osb", name="osb", bufs=3)
                for nn in range(2):
                    ps = psum.tile([128, 512], f32, tag="fill", name="outps", bufs=2)
                    for hd in range(2):
                        nc.tensor.matmul(
                            ps,
                            lhsT=mixT_sb[hd][:, qb * 128 : (qb + 1) * 128],
                            rhs=wout_sb[hd][:, nn * 512 : (nn + 1) * 512],
                            start=(hd == 0),
                            stop=(hd == 1),
                        )
                        yield
                    dst = o_sb[:, nn * 512 : (nn + 1) * 512]
                    if evict_act:
                        nc.scalar.activation(out=dst, in_=ps, func=Copy)
                    else:
                        nc.vector.tensor_copy(out=dst, in_=ps)
                    qs_rot = (nc.sync, nc.gpsimd)
                    odma = qs_rot[state["odma"] % len(qs_rot)]
                    state["odma"] += 1
                    odma.dma_start(
                        out=out[qb * 128 : (qb + 1) * 128, nn * 512 : (nn + 1) * 512],
                        in_=o_sb[:, nn * 512 : (nn + 1) * 512],
                    )

            fillers = []

            def enqueue(gen):
                fillers.append(gen)

            def fill_step(n):
                for _ in range(n):
                    while fillers:
                        try:
                            next(fillers[0])
                            break
                        except StopIteration:
                            fillers.pop(0)
                    else:
                        return

            def run_block(gen):
                for _ in gen:
                    pass

            def flush():
                while fillers:
                    run_block(fillers.pop(0))

            def attention(hp, qs, pops=1):
                """One head-pair (heads 2hp, 2hp+1) x one 512-col q group."""
                he, ho = 2 * hp, 2 * hp + 1
                qt, kt = qk_sb[hp], qk_sb[2 + hp]
                gt = g_sb[hp]
                kbmax = qs // 128 + 3
                mix_o = psum.tile([65, 512], f32, tag="mix", name="mixo", bufs=2)
                mix_e = psum.tile([65, 512], f32, tag="mix", name="mixe", bufs=2)

                def pv(kb, pband):
                    # PV: interior k-blocks skip their zero q-prefix; first and
                    # last stay full-width so PSUM start/stop cover everything.
                    p_ = max(0, 128 * kb - qs)
                    pvs = 0 if kb in (0, kbmax) else p_
                    nc.tensor.matmul(
                        mix_e[:, pvs:],
                        lhsT=v_sb[kb][:, 65 * he : 65 * he + 65],
                        rhs=pband[:, pvs:512],
                        start=(kb == 0),
                        stop=(kb == kbmax),
                    )
                    nc.tensor.matmul(
                        mix_o[:, pvs:],
                        lhsT=v_sb[kb][:, 65 * ho : 65 * ho + 65],
                        rhs=pband[:, 512 + pvs : 1024],
                        start=(kb == 0),
                        stop=(kb == kbmax),
                    )

                pend = None  # (kb, pband) whose PV is deferred one step
                for kb in range(kbmax + 1):
                    k0 = 128 * kb
                    p_ = max(0, k0 - qs)
                    # two-region score band: region0 (cols 0-511) = even head,
                    # region1 (cols 512-1023) = odd head; the 64x128 row-tiled
                    # matmuls (tile rows 0 / 64) execute concurrently.
                    sb = psum.tile([128, 1024], f32, tag="sband", name="sband", bufs=2)
                    nc.tensor.matmul(
                        sb[:, p_:512],
                        lhsT=kt[0:64, k0 : k0 + 128],
                        rhs=qt[0:64, qs + p_ : qs + 512],
                        start=True,
                        stop=True,
                    )
                    nc.tensor.matmul(
                        sb[:, 512 + p_ : 1024],
                        lhsT=kt[64:128, k0 : k0 + 128],
                        rhs=qt[64:128, qs + p_ : qs + 512],
                        start=True,
                        stop=True,
                    )
                    pexp = work.tile([128, 1024], bf16, tag="pexp", name="pexp")
                    nc.scalar.activation(out=pexp[:, p_:1024], in_=sb[:, p_:1024], func=Exp)
                    # previous step's PV goes into the PE FIFO *here* -- its
                    # pband is already ready, so the PE never stalls on it and
                    # the next QK (and therefore the Act exp stream) is never
                    # blocked behind this step's exp->Gmul round trip.
                    if pend is not None:
                        pv(*pend)
                    fill_step(pops)
                    # one G-multiply for both heads: slot a=0 -> even head's
                    # strip, a=1 -> odd head's strip (g_sb packs [even|odd]).
                    o0 = 512 + qs - k0
                    gv = AP(
                        tensor=gt.tensor,
                        offset=gt.offset + o0,
                        ap=[list(gt.ap[0]), [GW, 2], [1, 512]],
                    )
                    pband = work.tile([128, 1024], bf16, tag="pband", name="pband")
                    pb3 = pband.rearrange("p (a b) -> p a b", b=512)
                    px3 = pexp.rearrange("p (a b) -> p a b", b=512)
                    nc.vector.tensor_mul(pb3, px3, gv)
                    fill_step(1)
                    pend = (kb, pband)
                pv(*pend)
                # close both heads: evict mix PSUM to SBUF (frees the bank),
                # then reciprocal-normalize rowsums into mixT.
                for h, mixp in ((he, mix_e), (ho, mix_o)):
                    ms = msbp.tile([65, 512], f32, tag="msb", name="msb")
                    nc.vector.tensor_copy(out=ms, in_=mixp)
                    if h == 3 and qs == 1536:
                        # Final chain gates the kernel tail: route it through
                        # the (idle by now) PE instead of DRAM DMA latency.
                        row_sb = work2.tile([1, 512], f32, tag="row", name="row")
                        nc.vector.tensor_copy(out=row_sb, in_=ms[64:65, :])
                        tp_ps = psum.tile([128, 1024], f32, tag="sband", name="tp", bufs=2)
                        for j in range(4):
                            nc.tensor.transpose(
                                tp_ps[:, j : j + 1],
                                row_sb[:, 128 * j : 128 * (j + 1)],
                                ident128[0:1, 0:1],
                            )
                        rc_sb = work2.tile([128, 4], f32, tag="rc", name="rc")
                        nc.vector.reciprocal(out=rc_sb, in_=tp_ps[:, 0:4])
                        ut_ps = psum.tile([128, 1024], f32, tag="sband", name="ut", bufs=2)
                        for j in range(4):
                            nc.tensor.transpose(
                                ut_ps[0:1, 128 * j : 128 * (j + 1)],
                                rc_sb[:, j : j + 1],
                                ident128,
                            )
                        rrow_sb = work2.tile([1, 512], f32, tag="rrow", name="rrow")
                        nc.vector.tensor_copy(out=rrow_sb, in_=ut_ps[0:1, 0:512])
                        r_ps = psum.tile([128, 1024], f32, tag="sband", name="rps", bufs=2)
                        nc.tensor.matmul(
                            r_ps[0:64, 0:512], lhsT=ones64, rhs=rrow_sb, start=True, stop=True
                        )
                        r_sb = work2.tile([64, 512], f32, tag="rbc", name="rbc")
                        nc.vector.tensor_copy(out=r_sb, in_=r_ps[0:64, 0:512])
                    else:
                        # DRAM bounce wraps [1,512] to [128,4] so the
                        # reciprocal runs on 128 lanes. Chains alternate
                        # SP/GpSimd DMA queues so consecutive heads overlap.
                        dma_eng = nc.sync if state["chain"] % 2 == 0 else nc.gpsimd
                        state["chain"] += 1
                        d_s = dpool.tile([1, 512], f32, tag="ds", name="ds")
                        dma_eng.dma_start(out=d_s, in_=ms[64:65, :])
                        rs_sb = work2.tile([128, 4], f32, tag="rs", name="rs")
                        dma_eng.dma_start(out=rs_sb, in_=d_s.rearrange("o (a b) -> (o a) b", a=128))
                        rc_sb = work2.tile([128, 4], f32, tag="rc", name="rc")
                        nc.vector.reciprocal(out=rc_sb, in_=rs_sb)
                        d_r = dpool.tile([1, 512], f32, tag="dr", name="dr")
                        dma_eng.dma_start(out=d_r.rearrange("o (a b) -> (o a) b", a=128), in_=rc_sb)
                        r_sb = work2.tile([64, 512], f32, tag="rbc", name="rbc")
                        dma_eng.dma_start(out=r_sb, in_=d_r.to_broadcast([64, 512]))
                    # 64-partition DVE ops may write either partition half
                    # (bank->quadrant routing), so odd heads write rows 64-127
                    pb = 64 * (h % 2)
                    nc.vector.tensor_mul(
                        mixT_sb[hp][pb : pb + 64, qs : qs + 512], ms[0:64, :], r_sb
                    )

            # ---------------- schedule (qs-major, per-step fillers) ----------
            # flush() before every segment that consumes queued filler output
            run_block(qk_pass(0, 0, evict_act=True))
            run_block(qk_pass(2, 0, evict_act=True))
            x_second_wave()
            for st in range(4):
                run_block(v_pass(st))
            enqueue(v_pass(4))
            enqueue(v_pass(5))
            attention(0, 0)
            flush()
            run_block(qk_pass(1, 0))
            run_block(qk_pass(3, 0))
            enqueue(v_pass(6))
            enqueue(v_pass(7))
            attention(1, 0)
            flush()  # V0-7 complete for qs=512
            for st in range(8, 12):
                enqueue(v_pass(st))
            enqueue(qk_pass(0, 1))
            enqueue(qk_pass(2, 1))
            attention(0, 512)
            attention(1, 512)
            flush()  # V8-11 + qk halves complete for qs=1024 (hp=0)
            enqueue(qk_pass(1, 1))
            enqueue(qk_pass(3, 1))
            attention(0, 1024)
            flush()  # hp=1 qk halves complete
            for st in range(12, 16):
                enqueue(v_pass(st))
            attention(1, 1024)
            flush()  # V12-15 complete for qs=1536
            for qb in range(0, 8):
                enqueue(out_proj(qb))
            attention(0, 1536, pops=2)
            attention(1, 1536, pops=2)
            # out_proj 8-11 held back: they keep the PE warm through the
            # final normalization chains (and their DMAs drain early).
            for qb in range(8, 12):
                enqueue(out_proj(qb))
            flush()
            for qb in range(12, 16):
                run_block(out_proj(qb, evict_act=(qb % 2 == 0)))
    nc.finalize()
    return nc


def host_prep(x, qkv, out_w, rpe_bias):
    """Build per-core input shards (all host work is O(N*S) or a transpose)."""
    x = np.asarray(x, np.float32)
    qkv = np.asarray(qkv, np.float32)
    out_w = np.asarray(out_w, np.float32)
    rpe_bias = np.asarray(rpe_bias, np.float32)

    xT = [np.ascontiguousarray(x[b].T).astype(BF16) for b in range(B)]  # [HID,S]

    shards = []
    for hg in range(4):
        hs = slice(hg * 4, hg * 4 + 4)
        wq = qkv[:, 0, hs, :].reshape(HID, 256) * (D ** -0.5)
        wk = qkv[:, 1, hs, :].reshape(HID, 256)
        w_qk = np.concatenate([wq, wk], axis=1).astype(BF16)

        w_v = np.zeros((HID, 260), np.float32)
        for i in range(4):
            w_v[:, i * 65 : i * 65 + 64] = qkv[:, 2, hg * 4 + i, :]
        w_v = w_v.astype(BF16)

        # per-head G strips packed per head-PAIR: [even | odd]
        g = np.zeros((HPC, 128, GW), np.float32)
        idx = np.arange(GW)[None, :] - 512 - np.arange(128)[:, None]
        valid = (idx >= 0) & (idx < S)
        for i in range(4):
            cum = np.cumsum(rpe_bias[hg * 4 + i])
            gh = np.exp(cum)
            g[i] = np.where(valid, gh[np.clip(idx, 0, S - 1)], 0.0)
        g2 = np.concatenate(
            [
                np.concatenate([g[0], g[1]], axis=1)[None],
                np.concatenate([g[2], g[3]], axis=1)[None],
            ]
        ).astype(BF16)

        w_o = out_w[hs].reshape(256, HID).astype(BF16)
        shards.append((w_qk, w_v, g2, w_o))

    in_maps = []
    for c in range(NCORES):
        b, hg = c // 4, c % 4
        w_qk, w_v, g2, w_o = shards[hg]
        in_maps.append(
            {"x_t": xT[b], "w_qk": w_qk, "w_v": w_v, "g_ext": g2, "w_out": w_o}
        )
    return in_maps


def run(in_maps, trace=False):
    from concourse.bass_utils import run_bass_kernel_spmd

    if "nc" not in _CACHE:
        _CACHE["nc"] = build_nc()
    nc = _CACHE["nc"]
    res = run_bass_kernel_spmd(nc, in_maps, core_ids=list(range(NCORES)), trace=trace)
    return res


def kernel(x, qkv, out_w, rpe_bias):
    in_maps = host_prep(x, qkv, out_w, rpe_bias)
    res = run(in_maps)
    parts = [np.asarray(res.results[c]["out"], np.float32) for c in range(NCORES)]
    out = np.stack(
        [
            parts[0] + parts[1] + parts[2] + parts[3],
            parts[4] + parts[5] + parts[6] + parts[7],
        ]
    ).astype(np.float32)
    return out


if __name__ == "__main__":
    nc = build_nc()
    print("built ok")


# revision 22
# speedup vs baseline: 1.1395x; 1.0599x over previous
"""Distributed causal multi-head attention with cumulative relative-position
bias for Trainium2 (8 NeuronCores).

Problem: x:[2,2048,1024], qkv:[1024,3,16,64], out_w:[16,64,1024],
rpe_bias:[16,2048] -> out:[2,2048,1024]

Sharding: data-parallel over batch (2) x tensor-parallel over head groups
(16 heads -> 4 groups of 4). Core c handles batch c//4, heads [4*(c%4), 4*(c%4)+4).
Each core emits a partial output [2048,1024] (bf16); the host sums the 4
head-group partials per batch (the "all-reduce" of the out projection).

Math tricks:
 - 1/sqrt(D) folded into the Q projection weights on host.
 - bias[i,j] = cumsum(rpe)[i-j] (i>=j, else -inf) is Toeplitz:
   exp(S + bias) = exp(S) * G where G[k_row, q_col] = exp(cum)[q-k] (0 above
   the diagonal). A single [128, 512+2048] bf16 strip per head serves every
   128-row band of the score matrix; the two heads of a pair are packed in
   one [128, 2*(512+2048)] strip so one strided AP covers both.
 - Scores are computed transposed (S^T[k,q] = K_h Q_h^T) so that P^T feeds
   the PV matmul directly (lhsT = V natural layout) with no transposes.
 - A ones-column appended to V yields softmax row-sums for free in the same
   matmul; normalization happens on the SBUF copy of mix^T.

Schedule (v6): attention processes HEAD-PAIRS: per 128-row k-block, the even
head's scores (partitions 0-63 of Q/K) and the odd head's (partitions 64-127)
are computed by two row-tiled 64x128 matmuls (tile rows 0 / 64) that execute
CONCURRENTLY in the PE array, into the two regions of a [128,1024] score
band (region0 = even head, region1 = odd head; different PSUM banks as
row-tiling requires).  One exp per band (stale/hole columns are finite --
the band PSUM pool is zeroed once -- and get G=0), one G-multiply via the
packed dual-head strided AP, then two serial PV matmuls (M=65, ones-column
rowsum).  mix PSUM is evicted to SBUF with a single [65,512] copy (same DVE
cost as the old [1,512] rowsum copy) so each mix bank recycles immediately
instead of living through the reciprocal chain.  Projection / out-projection
matmuls are interleaved as FILLER STEPS inside the attention loop.
Input DMAs are spread across the GpSimd/Tensor/Sync/Vector/Scalar queues in
consumption order so the first projection starts ~5us in; output DMAs are
split per 512-column half and alternate the Sync/GpSimd queues.
"""

import sys

if "/opt/trn_rl_repo" not in sys.path:
    sys.path.insert(0, "/opt/trn_rl_repo")

import numpy as np
import ml_dtypes

B, S, HID, NH, D = 2, 2048, 1024, 16, 64
NCORES = 8
HPC = 4  # heads per core
KB = 16  # 128-row k blocks
GW = 512 + S  # per-head G strip width
BF16 = ml_dtypes.bfloat16

_CACHE = {}


def build_nc():
    import concourse.mybir as mybir
    from concourse import bacc
    from concourse.tile import TileContext
    from concourse.ap import AP

    f32 = mybir.dt.float32
    bf16 = mybir.dt.bfloat16
    Exp = mybir.ActivationFunctionType.Exp
    Copy = mybir.ActivationFunctionType.Copy

    nc = bacc.Bacc()

    x_t = nc.declare_dram_parameter("x_t", [HID, S], bf16, isOutput=False)
    w_qk = nc.declare_dram_parameter("w_qk", [HID, 512], bf16, isOutput=False)
    w_v = nc.declare_dram_parameter("w_v", [HID, 260], bf16, isOutput=False)
    g_ext = nc.declare_dram_parameter("g_ext", [2, 128, 2 * GW], bf16, isOutput=False)
    w_out = nc.declare_dram_parameter("w_out", [256, HID], bf16, isOutput=False)
    out = nc.declare_dram_parameter("out", [S, HID], bf16, isOutput=True)

    with TileContext(nc) as tc:
        with (
            tc.tile_pool(name="persist", bufs=1) as persist,
            tc.tile_pool(name="work", bufs=3) as work,
            tc.tile_pool(name="work2", bufs=2) as work2,
            tc.tile_pool(name="msbp", bufs=3) as msbp,
            tc.tile_pool(name="dram", bufs=2, space="DRAM") as dpool,
            tc.tile_pool(name="psum", bufs=2, space="PSUM") as psum,
        ):
            # warm the exp activation-table during the input DMA wait
            warm = persist.tile([1, 8], f32, tag="warm", name="warm")
            nc.vector.memset(warm, 0.0)
            nc.scalar.activation(out=warm, in_=warm, func=Exp)

            # ---- input DMAs: 3 queues (GpSimd/Sync/Scalar), consumption order
            # First projection pass (qk_pass mt=0,2) needs all wqk tiles +
            # x cols [0,1024); V st0-7 needs x cols [0,1024).  Queues:
            #   GpSimd: wqk 0-3, x0-3 column blocks in order, wout
            #   Sync:   wqk 4-7, x4-7 column blocks in order, g2[1] 2nd half
            #   Scalar: wv 0-7, g2[0], g2[1] first half
            wqk_sb, xt_sb = [], []
            for i in range(8):
                tw = persist.tile([128, 512], bf16, tag=f"wqk{i}", name=f"wqk{i}")
                wqk_sb.append(tw)
                tx = persist.tile([128, S], bf16, tag=f"xt{i}", name=f"xt{i}")
                xt_sb.append(tx)
            g_sb = [
                persist.tile([128, 2 * GW], bf16, tag=f"g{hp}", name=f"g{hp}")
                for hp in range(2)
            ]
            wv_sb = [
                persist.tile([128, 260], bf16, tag=f"wv{i}", name=f"wv{i}")
                for i in range(8)
            ]
            wout_sb = [
                persist.tile([128, HID], bf16, tag=f"wout{i}", name=f"wout{i}")
                for i in range(2)
            ]
            for i in range(4):
                nc.gpsimd.dma_start(out=wqk_sb[i], in_=w_qk[i * 128 : (i + 1) * 128, :])
                nc.sync.dma_start(
                    out=wqk_sb[4 + i], in_=w_qk[(4 + i) * 128 : (5 + i) * 128, :]
                )
            for c in range(2):
                for i in range(4):
                    nc.gpsimd.dma_start(
                        out=xt_sb[i][:, 512 * c : 512 * (c + 1)],
                        in_=x_t[i * 128 : (i + 1) * 128, 512 * c : 512 * (c + 1)],
                    )
                    nc.sync.dma_start(
                        out=xt_sb[4 + i][:, 512 * c : 512 * (c + 1)],
                        in_=x_t[(4 + i) * 128 : (5 + i) * 128, 512 * c : 512 * (c + 1)],
                    )
            for i in range(8):
                nc.scalar.dma_start(out=wv_sb[i], in_=w_v[i * 128 : (i + 1) * 128, :])
            nc.scalar.dma_start(out=g_sb[0], in_=g_ext[0])

            def x_second_wave():
                # issued after the first projection passes are emitted so the
                # transfers queue behind the startup-critical ones.
                for c in range(2, 4):
                    for i in range(4):
                        nc.gpsimd.dma_start(
                            out=xt_sb[i][:, 512 * c : 512 * (c + 1)],
                            in_=x_t[i * 128 : (i + 1) * 128, 512 * c : 512 * (c + 1)],
                        )
                        nc.sync.dma_start(
                            out=xt_sb[4 + i][:, 512 * c : 512 * (c + 1)],
                            in_=x_t[(4 + i) * 128 : (5 + i) * 128, 512 * c : 512 * (c + 1)],
                        )
                nc.sync.dma_start(out=g_sb[1][:, 0:GW], in_=g_ext[1][:, 0:GW])
                nc.gpsimd.dma_start(out=g_sb[1][:, GW : 2 * GW], in_=g_ext[1][:, GW : 2 * GW])
                for i in range(2):
                    nc.gpsimd.dma_start(
                        out=wout_sb[i], in_=w_out[i * 128 : (i + 1) * 128, :]
                    )

            qk_sb = [persist.tile([128, S], bf16, tag=f"qk{mt}", name=f"qk{mt}") for mt in range(4)]
            v_sb = [persist.tile([128, 260], bf16, tag=f"v{st}", name=f"v{st}") for st in range(KB)]
            for st in range(KB):
                ones_cols = v_sb[st].rearrange("p (h c) -> p h c", c=65)
                nc.vector.memset(ones_cols[:, :, 64:65], 1.0)
            mixT_sb = [persist.tile([128, S], bf16, tag=f"mixT{i}", name=f"mixT{i}") for i in range(2)]

            # pexp buffers rotate; stale prefix columns are multiplied by G=0,
            # so they only need to hold FINITE values -> zero them once.
            for i in range(3):
                t = work.tile([128, 1024], bf16, tag="pexp", name="pexpinit")
                nc.vector.memset(t, 0.0)
            # the score-band PSUM pool is exp'ed across hole columns the QK
            # matmuls did not write this rotation; zero it once so stale bits
            # are never NaN/Inf patterns.
            for i in range(2):
                t = psum.tile([128, 1024], f32, tag="sband", name="sbinit", bufs=2)
                nc.vector.memset(t, 0.0)

            # identity + ones for the PE-based final normalization chain
            from concourse.masks import make_identity

            ident128 = persist.tile([128, 128], f32, tag="ident", name="ident")
            make_identity(nc, ident128)
            ones64 = persist.tile([1, 64], f32, tag="ones64", name="ones64")
            nc.vector.memset(ones64, 1.0)

            state = {"chain": 0, "odma": 0}

            def qk_pass(mt, half, evict_act=False):
                """Generator: one yield per matmul; eviction per 512-half."""
                for c in range(2):
                    ps = psum.tile([128, 512], f32, tag="fill", name="qkps", bufs=2)
                    for xc in range(8):
                        nc.tensor.matmul(
                            ps,
                            lhsT=wqk_sb[xc][:, mt * 128 : (mt + 1) * 128],
                            rhs=xt_sb[xc][:, half * 1024 + c * 512 : half * 1024 + (c + 1) * 512],
                            start=(xc == 0),
                            stop=(xc == 7),
                        )
                        yield
                    dst = qk_sb[mt][:, half * 1024 + c * 512 : half * 1024 + (c + 1) * 512]
                    if evict_act:
                        nc.scalar.activation(out=dst, in_=ps, func=Copy)
                    else:
                        nc.vector.tensor_copy(out=dst, in_=ps)

            def v_pass(st):
                ps = psum.tile([128, 512], f32, tag="fill", name="vps", bufs=2)[:, :260]
                for xc in range(8):
                    nc.tensor.matmul(
                        ps,
                        lhsT=xt_sb[xc][:, st * 128 : (st + 1) * 128],
                        rhs=wv_sb[xc],
                        start=(xc == 0),
                        stop=(xc == 7),
                    )
                    yield
                # strided eviction skips the ones columns (memset once at
                # startup; the projection would write zeros over them)
                v3 = v_sb[st].rearrange("p (h c) -> p h c", c=65)
                ps3 = ps.rearrange("p (h c) -> p h c", c=65)
                nc.vector.tensor_copy(out=v3[:, :, 0:64], in_=ps3[:, :, 0:64])

            def out_proj(qb, evict_act=False):
                o_sb = work2.tile([128, HID], bf16, tag="osb", name="osb", bufs=3)
                for nn in range(2):
                    ps = psum.tile([128, 512], f32, tag="fill", name="outps", bufs=2)
                    for hd in range(2):
                        nc.tensor.matmul(
                            ps,
                            lhsT=mixT_sb[hd][:, qb * 128 : (qb + 1) * 128],
                            rhs=wout_sb[hd][:, nn * 512 : (nn + 1) * 512],
                            start=(hd == 0),
                            stop=(hd == 1),
                        )
                        yield
                    dst = o_sb[:, nn * 512 : (nn + 1) * 512]
                    if evict_act:
                        nc.scalar.activation(out=dst, in_=ps, func=Copy)
                    else:
                        nc.vector.tensor_copy(out=dst, in_=ps)
                    qs_rot = (nc.sync, nc.gpsimd)
                    odma = qs_rot[state["odma"] % len(qs_rot)]
                    state["odma"] += 1
                    odma.dma_start(
                        out=out[qb * 128 : (qb + 1) * 128, nn * 512 : (nn + 1) * 512],
                        in_=o_sb[:, nn * 512 : (nn + 1) * 512],
                    )

            fillers = []

            def enqueue(gen):
                fillers.append(gen)

            def fill_step(n):
                for _ in range(n):
                    while fillers:
                        try:
                            next(fillers[0])
                            break
                        except StopIteration:
                            fillers.pop(0)
                    else:
                        return

            def run_block(gen):
                for _ in gen:
                    pass

            def flush():
                while fillers:
                    run_block(fillers.pop(0))

            def attention(hp, qs, pops=1):
                """One head-pair (heads 2hp, 2hp+1) x one 512-col q group."""
                he, ho = 2 * hp, 2 * hp + 1
                qt, kt = qk_sb[hp], qk_sb[2 + hp]
                gt = g_sb[hp]
                kbmax = qs // 128 + 3
                mix_o = psum.tile([65, 512], f32, tag="mix", name="mixo", bufs=2)
                mix_e = psum.tile([65, 512], f32, tag="mix", name="mixe", bufs=2)

                def pv(kb, pband):
                    # PV: interior k-blocks skip their zero q-prefix; first and
                    # last stay full-width so PSUM start/stop cover everything.
                    p_ = max(0, 128 * kb - qs)
                    pvs = 0 if kb in (0, kbmax) else p_
                    nc.tensor.matmul(
                        mix_e[:, pvs:],
                        lhsT=v_sb[kb][:, 65 * he : 65 * he + 65],
                        rhs=pband[:, pvs:512],
                        start=(kb == 0),
                        stop=(kb == kbmax),
                    )
                    nc.tensor.matmul(
                        mix_o[:, pvs:],
                        lhsT=v_sb[kb][:, 65 * ho : 65 * ho + 65],
                        rhs=pband[:, 512 + pvs : 1024],
                        start=(kb == 0),
                        stop=(kb == kbmax),
                    )

                pend = None  # (kb, pband) whose PV is deferred one step
                for kb in range(kbmax + 1):
                    k0 = 128 * kb
                    p_ = max(0, k0 - qs)
                    # two-region score band: region0 (cols 0-511) = even head,
                    # region1 (cols 512-1023) = odd head; the 64x128 row-tiled
                    # matmuls (tile rows 0 / 64) execute concurrently.
                    sb = psum.tile([128, 1024], f32, tag="sband", name="sband", bufs=2)
                    nc.tensor.matmul(
                        sb[:, p_:512],
                        lhsT=kt[0:64, k0 : k0 + 128],
                        rhs=qt[0:64, qs + p_ : qs + 512],
                        start=True,
                        stop=True,
                    )
                    nc.tensor.matmul(
                        sb[:, 512 + p_ : 1024],
                        lhsT=kt[64:128, k0 : k0 + 128],
                        rhs=qt[64:128, qs + p_ : qs + 512],
                        start=True,
                        stop=True,
                    )
                    pexp = work.tile([128, 1024], bf16, tag="pexp", name="pexp")
                    nc.scalar.activation(out=pexp[:, p_:1024], in_=sb[:, p_:1024], func=Exp)
                    # previous step's PV goes into the PE FIFO *here* -- its
                    # pband is already ready, so the PE never stalls on it and
                    # the next QK (and therefore the Act exp stream) is never
                    # blocked behind this step's exp->Gmul round trip.
                    if pend is not None:
                        pv(*pend)
                    fill_step(pops)
                    # one G-multiply for both heads: slot a=0 -> even head's
                    # strip, a=1 -> odd head's strip (g_sb packs [even|odd]).
                    o0 = 512 + qs - k0
                    gv = AP(
                        tensor=gt.tensor,
                        offset=gt.offset + o0,
                        ap=[list(gt.ap[0]), [GW, 2], [1, 512]],
                    )
                    pband = work.tile([128, 1024], bf16, tag="pband", name="pband")
                    pb3 = pband.rearrange("p (a b) -> p a b", b=512)
                    px3 = pexp.rearrange("p (a b) -> p a b", b=512)
                    nc.vector.tensor_mul(pb3, px3, gv)
                    fill_step(1)
                    pend = (kb, pband)
                pv(*pend)
                # close both heads: evict mix PSUM to SBUF first (frees both
                # banks for the next group / the reciprocal broadcast), then
                # reciprocal-normalize rowsums into mixT.
                ms_e = msbp.tile([65, 512], f32, tag="msb", name="msb")
                nc.vector.tensor_copy(out=ms_e, in_=mix_e)
                ms_o = msbp.tile([65, 512], f32, tag="msb", name="msb")
                nc.vector.tensor_copy(out=ms_o, in_=mix_o)
                for h, ms in ((he, ms_e), (ho, ms_o)):
                    if qs == 1536 and hp == 1:
                        # Final chains gate the kernel tail: approximate
                        # reciprocal (51 ULP -- plenty for softmax norms) and a
                        # ones-matmul partition broadcast instead of the ~6.5us
                        # DRAM DMA bounce.
                        row_sb = work2.tile([1, 512], f32, tag="row", name="row")
                        nc.vector.tensor_copy(out=row_sb, in_=ms[64:65, :])
                        rr_sb = work2.tile([1, 512], f32, tag="rrow", name="rrow")
                        nc.vector.reciprocal_approx_fast(out=rr_sb, in_=row_sb)
                        r_ps = psum.tile([65, 512], f32, tag="mix", name="rps", bufs=2)
                        nc.tensor.matmul(
                            r_ps[0:64, :], lhsT=ones64, rhs=rr_sb, start=True, stop=True
                        )
                        r_sb = work2.tile([64, 512], f32, tag="rbc", name="rbc")
                        nc.vector.tensor_copy(out=r_sb, in_=r_ps[0:64, :])
                    else:
                        # DRAM bounce wraps [1,512] to [128,4] so the
                        # reciprocal runs on 128 lanes. Chains alternate
                        # SP/GpSimd DMA queues so consecutive heads overlap.
                        dma_eng = nc.sync if state["chain"] % 2 == 0 else nc.gpsimd
                        state["chain"] += 1
                        d_s = dpool.tile([1, 512], f32, tag="ds", name="ds")
                        dma_eng.dma_start(out=d_s, in_=ms[64:65, :])
                        rs_sb = work2.tile([128, 4], f32, tag="rs", name="rs")
                        dma_eng.dma_start(out=rs_sb, in_=d_s.rearrange("o (a b) -> (o a) b", a=128))
                        rc_sb = work2.tile([128, 4], f32, tag="rc", name="rc")
                        nc.vector.reciprocal(out=rc_sb, in_=rs_sb)
                        d_r = dpool.tile([1, 512], f32, tag="dr", name="dr")
                        dma_eng.dma_start(out=d_r.rearrange("o (a b) -> (o a) b", a=128), in_=rc_sb)
                        r_sb = work2.tile([64, 512], f32, tag="rbc", name="rbc")
                        dma_eng.dma_start(out=r_sb, in_=d_r.to_broadcast([64, 512]))
                    # 64-partition DVE ops may write either partition half
                    # (bank->quadrant routing), so odd heads write rows 64-127
                    pb = 64 * (h % 2)
                    nc.vector.tensor_mul(
                        mixT_sb[hp][pb : pb + 64, qs : qs + 512], ms[0:64, :], r_sb
                    )

            # ---------------- schedule (qs-major, per-step fillers) ----------
            # flush() before every segment that consumes queued filler output
            run_block(qk_pass(0, 0, evict_act=True))
            run_block(qk_pass(2, 0, evict_act=True))
            x_second_wave()
            for st in range(4):
                run_block(v_pass(st))
            enqueue(v_pass(4))
            enqueue(v_pass(5))
            attention(0, 0)
            flush()
            run_block(qk_pass(1, 0))
            run_block(qk_pass(3, 0))
            enqueue(v_pass(6))
            enqueue(v_pass(7))
            attention(1, 0)
            flush()  # V0-7 complete for qs=512
            for st in range(8, 12):
                enqueue(v_pass(st))
            enqueue(qk_pass(0, 1))
            enqueue(qk_pass(2, 1))
            attention(0, 512)
            attention(1, 512)
            flush()  # V8-11 + qk halves complete for qs=1024 (hp=0)
            enqueue(qk_pass(1, 1))
            enqueue(qk_pass(3, 1))
            attention(0, 1024)
            flush()  # hp=1 qk halves complete
            for st in range(12, 16):
                enqueue(v_pass(st))
            attention(1, 1024)
            flush()  # V12-15 complete for qs=1536
            for qb in range(0, 12):
                enqueue(out_proj(qb))
            attention(0, 1536, pops=2)
            attention(1, 1536, pops=2)
            flush()
            for qb in range(12, 16):
                run_block(out_proj(qb, evict_act=(qb % 2 == 0)))
    nc.finalize()
    return nc


def host_prep(x, qkv, out_w, rpe_bias):
    """Build per-core input shards (all host work is O(N*S) or a transpose)."""
    x = np.asarray(x, np.float32)
    qkv = np.asarray(qkv, np.float32)
    out_w = np.asarray(out_w, np.float32)
    rpe_bias = np.asarray(rpe_bias, np.float32)

    xT = [np.ascontiguousarray(x[b].T).astype(BF16) for b in range(B)]  # [HID,S]

    shards = []
    for hg in range(4):
        hs = slice(hg * 4, hg * 4 + 4)
        wq = qkv[:, 0, hs, :].reshape(HID, 256) * (D ** -0.5)
        wk = qkv[:, 1, hs, :].reshape(HID, 256)
        w_qk = np.concatenate([wq, wk], axis=1).astype(BF16)

        w_v = np.zeros((HID, 260), np.float32)
        for i in range(4):
            w_v[:, i * 65 : i * 65 + 64] = qkv[:, 2, hg * 4 + i, :]
        w_v = w_v.astype(BF16)

        # per-head G strips packed per head-PAIR: [even | odd]
        g = np.zeros((HPC, 128, GW), np.float32)
        idx = np.arange(GW)[None, :] - 512 - np.arange(128)[:, None]
        valid = (idx >= 0) & (idx < S)
        for i in range(4):
            cum = np.cumsum(rpe_bias[hg * 4 + i])
            gh = np.exp(cum)
            g[i] = np.where(valid, gh[np.clip(idx, 0, S - 1)], 0.0)
        g2 = np.concatenate(
            [
                np.concatenate([g[0], g[1]], axis=1)[None],
                np.concatenate([g[2], g[3]], axis=1)[None],
            ]
        ).astype(BF16)

        w_o = out_w[hs].reshape(256, HID).astype(BF16)
        shards.append((w_qk, w_v, g2, w_o))

    in_maps = []
    for c in range(NCORES):
        b, hg = c // 4, c % 4
        w_qk, w_v, g2, w_o = shards[hg]
        in_maps.append(
            {"x_t": xT[b], "w_qk": w_qk, "w_v": w_v, "g_ext": g2, "w_out": w_o}
        )
    return in_maps


def run(in_maps, trace=False):
    from concourse.bass_utils import run_bass_kernel_spmd

    if "nc" not in _CACHE:
        _CACHE["nc"] = build_nc()
    nc = _CACHE["nc"]
    res = run_bass_kernel_spmd(nc, in_maps, core_ids=list(range(NCORES)), trace=trace)
    return res


def kernel(x, qkv, out_w, rpe_bias):
    in_maps = host_prep(x, qkv, out_w, rpe_bias)
    res = run(in_maps)
    parts = [np.asarray(res.results[c]["out"], np.float32) for c in range(NCORES)]
    out = np.stack(
        [
            parts[0] + parts[1] + parts[2] + parts[3],
            parts[4] + parts[5] + parts[6] + parts[7],
        ]
    ).astype(np.float32)
    return out


if __name__ == "__main__":
    nc = build_nc()
    print("built ok")
